# revision 38
# baseline (speedup 1.0000x reference)
"""Trainium2 8-core kernel for a dense pre-norm transformer block.

Reference: h=LN1(x); qkv=h@w_qkv; causal MHA (16 heads, Dh=64);
x+=o@w_out; h2=LN2(x); x+=gelu(h2@w1+b1)@w2+b2.

Sharding (Megatron TP-8 + sequence-parallel residual):
  - heads 2c,2c+1 on core c (w_qkv column-shard)
  - residual stream token-sharded: core c owns the four strided pieces
    {q*1024 + c*128 .. +128}, q=0..3.
  - out-projection: per-quarter AllToAll of the (normalized) per-head
    attention outputs moves activations (256KB) instead of partial
    products (2MB ReduceScatter); each core then out-projects only its
    own tokens against the full w_out.
  - MLP token-sharded: each core runs the full-width MLP on its 512
    tokens with fp8 DoubleRow matmuls; w2 (fp8, k-paired) is SBUF-
    resident (prefetched during the startup collective window), w1 is
    streamed in 512-col chunks.
  - MLP1 computed hidden-major (w1 as lhsT) so gelu writes g1^T
    directly - no PE transposes between MLP1 and MLP2.
  - LN1 stats shard-local + tiny AllGather; LN scales folded into
    weights host-side; mean/bias corrections applied as K=2 outer-
    product matmuls + gpsimd partition_broadcast (no Act involvement).
  - softmax division via gpsimd partition_broadcast of 1/den.
"""
import numpy as np

import concourse.bass as bass
import concourse.mybir as mybir
import concourse.tile as tile
from concourse import bacc
from concourse import bass_utils
from concourse.masks import make_identity

F32 = mybir.dt.float32
BF = mybir.dt.bfloat16
F8 = mybir.dt.float8e4
AF = mybir.ActivationFunctionType
DR = mybir.MatmulPerfMode.DoubleRow

NCORES = 8
B, L, D = 2, 2048, 1024
T = B * L              # 4096 tokens
TSH = T // NCORES      # 512 tokens per core (4 pieces of 128)
DH = 64                # head dim
HL = 2                 # heads per core
DLOC = HL * DH         # 128 local head features
LN_EPS = 1e-5
NT = T // 512          # 8 token tiles of 512
ND = D // 128          # 8 feature chunks
WS = 64.0              # host-side fp8 weight scale (power of two)

_CACHE = {}


def build():
    if "nc" in _CACHE:
        return _CACHE["nc"]
    nc = bacc.Bacc("TRN2", target_bir_lowering=False, debug=False,
                   num_devices=NCORES)

    xt_in = nc.dram_tensor("xt", [D, T], F8, kind="ExternalInput")
    xsh_in = nc.dram_tensor("xsh", [TSH, D], BF, kind="ExternalInput")
    wqkv_in = nc.dram_tensor("wqkv", [ND // 2, 128, 2, 3 * DLOC], F8,
                             kind="ExternalInput")
    nws_in = nc.dram_tensor("nws", [3 * DLOC, 1], F32, kind="ExternalInput")
    bqkv_in = nc.dram_tensor("bqkv", [3 * DLOC, 1], F32, kind="ExternalInput")
    woutp_in = nc.dram_tensor("woutp", [ND // 2, 128, 2, D], BF,
                              kind="ExternalInput")
    w1b_in = nc.dram_tensor("w1b", [128, ND, 4096], BF,
                            kind="ExternalInput")
    b1c_in = nc.dram_tensor("b1c", [128, 32], F32, kind="ExternalInput")
    w2p_in = nc.dram_tensor("w2p", [16, 128, 2, D], F8,
                            kind="ExternalInput")
    b2b_in = nc.dram_tensor("b2b", [128, D], BF, kind="ExternalInput")
    mask_in = nc.dram_tensor("mask", [128, 128], BF, kind="ExternalInput")
    out_ext = nc.dram_tensor("out", [TSH, D], F32, kind="ExternalOutput")

    rg = [list(range(NCORES))]

    with tile.TileContext(nc) as tc:
        with (
            tc.tile_pool(name="const", bufs=1) as const,
            tc.tile_pool(name="wpool", bufs=1) as wpool,
            tc.tile_pool(name="dram", bufs=1, space="DRAM") as dram,
            tc.tile_pool(name="psum", bufs=1, space="PSUM") as psum,
        ):
            # ---- DRAM scratch for collectives ----
            warm_in = dram.tile([8, 16], BF)
            warm_out = dram.tile([64, 16], BF, addr_space="Shared")
            a2a_in = [dram.tile([D, 128], BF, name=f"a2a_in{q}")
                      for q in range(4)]
            a2a_out = [dram.tile([D, 128], BF, name=f"a2a_out{q}")
                       for q in range(4)]

            # ---- PSUM tags (single pool, 8 banks total) ----
            # tagA: [128,2,512] f32 (2 banks) x2   - score pairs, fin-P,
            #        qkv praw, oproj, MLP pm (sliced [:,0,:])
            # tagB: [65,512] f32 (1 bank)  x2      - attention po
            # tagC: [128,512] f32 (1 bank) x2      - transposes, diag pst
            def tA():
                return psum.tile([128, 2, 512], F32, tag="tagA", bufs=2,
                                 name="tA")

            def tC():
                return psum.tile([128, 512], F32, tag="tagC", bufs=2,
                                 name="tC")

            def tCbf(shape):
                return psum.tile(shape, BF, tag="tagC", bufs=2, name="tCbf")

            # ---- constants ----
            ident_bf = const.tile([128, 128], BF)
            make_identity(nc, ident_bf[:])
            c64 = const.tile([128, 1], F32)
            nc.vector.memset(c64[:], 1.0 / WS)
            eps128 = const.tile([128, 1], F32)
            nc.vector.memset(eps128[:], LN_EPS)
            mask_sb = const.tile([128, 128], BF)
            # packed LN1 stat rows: row0 = mean*rstd per tile (free-
            # packed), row1 = ones; rstd rows likewise free-packed
            ones128_f8 = const.tile([128, 128], F8)
            nc.vector.memset(ones128_f8[:], 1.0)
            ones128_bf = const.tile([128, 128], BF)
            nc.vector.memset(ones128_bf[:], 1.0)
            c1024 = const.tile([128, 1], F32)
            nc.vector.memset(c1024[:], 1.0 / 1024.0)
            eps4k = const.tile([128, 1], F32)
            nc.vector.memset(eps4k[:], LN_EPS * WS * WS)

            # ---- weights resident in SBUF ----
            wqkv_sb = []
            for p in range(ND // 2):
                wt = wpool.tile([128, 2, 3 * DLOC], F8, name=f"wqkv{p}")
                nc.sync.dma_start(wt[:], wqkv_in.ap()[p])
                wqkv_sb.append(wt)
            nws_sb = []   # [128, 1] per m: -colsum(w8) column
            bq_sb = []    # [128, 1] per m: bias column
            for m in range(3):
                t_ = wpool.tile([128, 1], F32, name=f"nws{m}")
                nc.scalar.dma_start(t_[:], nws_in.ap()[m * 128:(m + 1) * 128])
                nws_sb.append(t_)
                b_ = wpool.tile([128, 1], F32, name=f"bq{m}")
                nc.scalar.dma_start(b_[:], bqkv_in.ap()[m * 128:(m + 1) * 128])
                bq_sb.append(b_)
            woutp_sb = wpool.tile([128, ND // 2, 2, D], BF)
            b1c_sb = wpool.tile([128, 32], F32)
            w2p_sb = wpool.tile([128, 16, 2, D], F8)
            b2b_sb = wpool.tile([128, D], BF)

            def load_small_weights():
                nc.scalar.dma_start(mask_sb[:], mask_in.ap())
                nc.scalar.dma_start(b1c_sb[:], b1c_in.ap())
                nc.scalar.dma_start(b2b_sb[:], b2b_in.ap())

            def load_late_weights():
                nc.gpsimd.dma_start(
                    woutp_sb[:],
                    woutp_in.ap().rearrange("c p t n -> p c t n"))
                nc.gpsimd.dma_start(
                    w2p_sb[:],
                    w2p_in.ap().rearrange("c p t n -> p c t n"))

            # persistent activations
            attn_pool_cm = tc.tile_pool(name="attn", bufs=1)
            attn_pool = attn_pool_cm.__enter__()
            qkvT = []
            for m in range(3):
                t_ = attn_pool.tile([128, T], F8 if m < 2 else BF,
                                    name=f"qkvT{m}")
                qkvT.append(t_)
            oT = attn_pool.tile([128, T], BF)

            resid_pool_cm = tc.tile_pool(name="resid", bufs=1)
            resid_pool = resid_pool_cm.__enter__()
            xsv = resid_pool.tile([128, 4, D], BF)    # x shard -> resid
            h2T = resid_pool.tile([128, ND, 512], BF)
            g1T = resid_pool.tile([128, 16, 2, 512], F8)

            oa_pool_cm = tc.tile_pool(name="oa", bufs=2)
            oa_pool = oa_pool_cm.__enter__()
            s4_cm = tc.tile_pool(name="s4", bufs=2)
            s4 = s4_cm.__enter__()

            # ========== stage 0: warmup AG + local all-token stats ==========
            s0_cm = tc.tile_pool(name="s0", bufs=2)
            s0 = s0_cm.__enter__()
            load_small_weights()
            wtile = s0.tile([8, 16], BF, tag="wtile", name="wtile")
            nc.vector.memset(wtile[:], 0.0)
            nc.scalar.dma_start(warm_in[:], wtile[:])
            nc.gpsimd.collective_compute(
                "AllGather", mybir.AluOpType.bypass, replica_groups=rg,
                ins=[warm_in[:].opt()], outs=[warm_out[:].opt()])
            # residual shard (bf16) load
            for q in range(4):
                nc.scalar.dma_start(
                    xsv[:, q, :], xsh_in.ap()[q * 128:(q + 1) * 128, :])

            # ================= stage 1: qkv matmuls =================
            s1_x_cm = tc.tile_pool(name="s1_x", bufs=2)
            s1_x = s1_x_cm.__enter__()
            praw_pool_cm = tc.tile_pool(name="s1_praw", bufs=6)
            praw_pool = praw_pool_cm.__enter__()
            s1f_cm = tc.tile_pool(name="s1f", bufs=2)
            s1f = s1f_cm.__enter__()

            def do_s1_mm(tt):
                xts = s1_x.tile([128, ND, 512], F8, tag="xts")
                nc.sync.dma_start(
                    xts[:],
                    xt_in.ap()[:, tt * 512:(tt + 1) * 512].rearrange(
                        "(d p) t -> p d t", p=128))
                sq = s0.tile([128, ND, 512], BF, tag="sq", name="sq")
                nc.scalar.activation(
                    sq[:].rearrange("p a f -> p (a f)"),
                    xts[:].rearrange("p a f -> p (a f)"), AF.Square)
                # qkv matmuls first (PE dense), stats sums after (tagC)
                psq = []
                for m in range(3):
                    ps_q = tA()
                    for p in range(ND // 2):
                        nc.tensor.matmul(
                            ps_q[:, 0, :],
                            wqkv_sb[p][:, :, m * 128:(m + 1) * 128],
                            xts[:, 2 * p:2 * p + 2, :],
                            start=(p == 0), stop=(p == ND // 2 - 1),
                            perf_mode=DR)
                    psq.append(ps_q)
                ps_s1 = tC()
                for d in range(ND):
                    nc.tensor.matmul(ps_s1[:], ones128_f8[:],
                                     xts[:, d, :], start=(d == 0),
                                     stop=(d == ND - 1))
                ps_s2 = tC()
                for d in range(ND):
                    nc.tensor.matmul(ps_s2[:], ones128_bf[:],
                                     sq[:, d, :], start=(d == 0),
                                     stop=(d == ND - 1))
                m_sb = s0.tile([128, 512], BF, tag="m_sb", name="m_sb")
                nc.vector.tensor_scalar(
                    out=m_sb[:], in0=ps_s1[:], scalar1=c1024[:],
                    scalar2=None, op0=mybir.AluOpType.mult)
                msq = s0.tile([128, 512], BF, tag="msq", name="msq")
                nc.vector.tensor_mul(msq[:], m_sb[:], m_sb[:])
                varf = s0.tile([128, 512], F32, tag="varf", name="varf")
                nc.vector.scalar_tensor_tensor(
                    out=varf[:], in0=ps_s2[:], scalar=c1024[:],
                    in1=msq[:], op0=mybir.AluOpType.mult,
                    op1=mybir.AluOpType.subtract)
                # R_b = rstd/64 = 1/sqrt(4096*var + 4096*eps)
                rb = s0.tile([128, 512], BF, tag="rb", name="rb")
                nc.scalar.activation(rb[:], varf[:],
                                     AF.Abs_reciprocal_sqrt,
                                     bias=eps4k[:], scale=WS * WS)
                mrb = s0.tile([128, 512], BF, tag="mrb", name="mrb")
                nc.vector.tensor_mul(mrb[:], m_sb[:], rb[:])
                win = slice(tt * 512, (tt + 1) * 512)
                for m in range(3):
                    pr = praw_pool.tile([128, 512], BF, tag="praw",
                                        name="pr")
                    nc.vector.tensor_mul(pr[:], psq[m][:, 0, :], rb[:])
                    t2 = s1f.tile([128, 512], BF, tag="t2", name="t2")
                    nc.vector.scalar_tensor_tensor(
                        out=t2[:], in0=mrb[:], scalar=nws_sb[m][:],
                        in1=pr[:], op0=mybir.AluOpType.mult,
                        op1=mybir.AluOpType.add)
                    nc.vector.tensor_scalar(
                        out=qkvT[m][:, win], in0=t2[:], scalar1=bq_sb[m][:],
                        scalar2=None, op0=mybir.AluOpType.add)

            vaug_cur = [None]

            def do_vaug(b, j):
                tok0 = b * L
                if j == 0:
                    vaug = attn_pool.tile([128, HL, L // 128, DH + 1], BF,
                                          tag="vaug", name="vaug", bufs=1)
                    nc.vector.memset(vaug[:, :, :, DH:DH + 1], 1.0)
                    vaug_cur[0] = vaug
                vaug = vaug_cur[0]
                for hl in range(HL):
                    hrow = hl * DH
                    vT_u = qkvT[2][hrow:hrow + DH, tok0:tok0 + L]
                    for kc in range(4 * j, 4 * j + 4):
                        pv = tCbf([128, DH])
                        nc.tensor.transpose(
                            pv[:],
                            vT_u[:, kc * 128:(kc + 1) * 128],
                            ident_bf[hrow:hrow + DH, hrow:hrow + DH])
                        nc.vector.tensor_copy(vaug[:, hl, kc, 0:DH], pv[:])
                return vaug

            def do_attn(b, j):
                tok0 = b * L
                vaug = do_vaug(b, j)
                nk = 4 * (j + 1)
                po = [psum.tile([65, 512], F32, tag="tagB", bufs=2,
                                name=f"po{hl}") for hl in range(HL)]
                win = slice(tok0 + j * 512, tok0 + (j + 1) * 512)
                for hl in range(HL):
                    hrow = hl * DH
                    qsl_full = qkvT[0][hrow:hrow + DH, win]
                    # --- paired sub-diagonal chunks ---
                    for mpair in range((nk - 4) // 2):
                        kc0 = 2 * mpair
                        pst2 = tA()
                        est2 = s1f.tile([128, 2, 512], BF, tag="est2",
                                        name="est2", bufs=2)
                        for r in range(2):
                            kc = kc0 + r
                            ksl = qkvT[1][hrow:hrow + DH,
                                          tok0 + kc * 128:
                                          tok0 + (kc + 1) * 128]
                            nc.tensor.matmul(pst2[:, r, :], ksl, qsl_full,
                                             start=True, stop=True,
                                             tile_position=(hrow, 0))
                        nc.scalar.activation(
                            est2[:].rearrange("p a f -> p (a f)"),
                            pst2[:].rearrange("p a f -> p (a f)"),
                            AF.Exp, scale=0.125)
                        for r in range(2):
                            nc.tensor.matmul(po[hl][:],
                                             vaug[:, hl, kc0 + r, :],
                                             est2[:, r, :],
                                             start=(kc0 + r == 0),
                                             stop=False)
                    # --- diagonal region: 4 chunks, partial width ---
                    for dm in range(4):
                        kc = nk - 4 + dm
                        col0 = 128 * dm
                        pst = tC()
                        nc.tensor.matmul(
                            pst[:, col0:],
                            qkvT[1][hrow:hrow + DH,
                                    tok0 + kc * 128:tok0 + (kc + 1) * 128],
                            qkvT[0][hrow:hrow + DH,
                                    tok0 + j * 512 + col0:
                                    tok0 + (j + 1) * 512],
                            start=True, stop=True, tile_position=(hrow, 0))
                        nc.vector.tensor_add(
                            pst[:, col0:col0 + 128],
                            pst[:, col0:col0 + 128], mask_sb[:])
                        estd = s1f.tile([128, 512], BF, tag="estd",
                                        name="estd", bufs=2)
                        nc.scalar.activation(estd[:, col0:], pst[:, col0:],
                                             AF.Exp, scale=0.125)
                        nc.tensor.matmul(po[hl][:, col0:],
                                         vaug[:, hl, kc, :],
                                         estd[:, col0:],
                                         start=(kc == 0),
                                         stop=(kc == nk - 1))
                # --- softmax division for this j-tile ---
                for hl in range(HL):
                    rec = s1f.tile([1, 512], F32, tag=f"rec_{hl}",
                                   name="rec", bufs=1)
                    nc.vector.tensor_copy(rec[:], po[hl][DH:DH + 1, :])
                    nc.vector.reciprocal_approx_fast(rec[:], rec[:])
                    rec64 = s1f.tile([64, 512], F32, tag=f"rec64_{hl}",
                                     name="rec64", bufs=1)
                    nc.gpsimd.partition_broadcast(
                        rec64[:], rec[:], channels=64)
                    nc.vector.tensor_mul(
                        oT[hl * DH:hl * DH + DH, win],
                        po[hl][0:DH, :], rec64[:])

            def do_a2a(q):
                """ship normalized oT slices for quarter q (tokens q*1024..)"""
                nc.gpsimd.dma_start(
                    a2a_in[q][:].rearrange("(c f) t -> f c t", f=128),
                    oT[:, q * 1024:(q + 1) * 1024].rearrange(
                        "f (c t) -> f c t", t=128))
                nc.gpsimd.collective_compute(
                    "AllToAll", mybir.AluOpType.bypass, replica_groups=rg,
                    ins=[a2a_in[q][:].opt()], outs=[a2a_out[q][:].opt()])

            def do_oproj_s4(q):
                """own-token out-projection for quarter q + residual + LN2."""
                oa = oa_pool.tile([128, ND, 128], BF, tag="oa", name="oa")
                nc.gpsimd.dma_start(
                    oa[:], a2a_out[q][:].rearrange("(c p) t -> p c t", p=128))
                for ncol in range(2):
                    pm = tA()
                    for mp in range(ND // 2):
                        nc.tensor.matmul(
                            pm[:, 0, :], oa[:, 2 * mp, :],
                            woutp_sb[:, mp, 0, ncol * 512:(ncol + 1) * 512],
                            start=(mp == 0), stop=False)
                        nc.tensor.matmul(
                            pm[:, 0, :], oa[:, 2 * mp + 1, :],
                            woutp_sb[:, mp, 1, ncol * 512:(ncol + 1) * 512],
                            start=False, stop=(mp == ND // 2 - 1))
                    cs = slice(ncol * 512, (ncol + 1) * 512)
                    nc.vector.tensor_add(xsv[:, q, cs], xsv[:, q, cs],
                                         pm[:, 0, :])
                # LN2 on my 128-token piece
                stats = s4.tile([128, 2, 6], F32, tag="stats", name="stats")
                x2v = xsv[:, q, :].rearrange("p (s f) -> p s f", s=2)
                for s in range(2):
                    nc.vector.bn_stats(stats[:, s, :], x2v[:, s, :])
                mv = s4.tile([128, 2], F32, tag="mv", name="mv")
                nc.vector.bn_aggr(mv[:], stats[:])
                rstd2 = s4.tile([128, 1], F32, tag="rstd2", name="rstd2")
                nc.scalar.activation(rstd2[:], mv[:, 1:2],
                                     AF.Abs_reciprocal_sqrt,
                                     bias=eps128[:])
                h2 = s4.tile([128, D], BF, tag="h2", name="h2")
                nc.vector.tensor_scalar(
                    out=h2[:], in0=xsv[:, q, :], scalar1=mv[:, 0:1],
                    scalar2=rstd2[:], op0=mybir.AluOpType.subtract,
                    op1=mybir.AluOpType.mult)
                for d in range(ND):
                    pt = tCbf([128, 128])
                    nc.tensor.transpose(
                        pt[:], h2[:, d * 128:(d + 1) * 128], ident_bf[:])
                    nc.vector.tensor_copy(
                        h2T[:, d, q * 128:(q + 1) * 128], pt[:])

            w1s_store = {}

            def do_mlp1(half):
                # MLP1 hidden-major: g1T[c] = gelu(w1[:,c].T @ h2T + b1[c])
                # w1 streamed bf16 in 512-col chunks; token halves for
                # tail overlap (half 0 = tokens 0:256, half 1 = 256:512)
                tw = slice(256 * half, 256 * half + 256)
                for c4 in range(8):
                    w1s = s4.tile([128, ND, 512], BF, tag="w1s",
                                  name="w1s", bufs=2)
                    nc.sync.dma_start(
                        w1s[:],
                        w1b_in.ap()[:, :, c4 * 512:(c4 + 1) * 512])
                    for ci in range(4):
                        c = 4 * c4 + ci
                        pm1 = tA()
                        for d in range(ND):
                            nc.tensor.matmul(
                                pm1[:, 0, tw],
                                w1s[:, d, ci * 128:(ci + 1) * 128],
                                h2T[:, d, tw],
                                start=(d == 0), stop=(d == ND - 1))
                        nc.scalar.activation(
                            g1T[:, c // 2, c % 2, tw], pm1[:, 0, tw],
                            AF.Gelu, bias=b1c_sb[:, c:c + 1])

            def do_mlp():
                # MLP2 + residual + out
                for tq in range(4):
                    ts_ = slice(tq * 128, (tq + 1) * 128)
                    for ncol in range(2):
                        pm2 = tA()
                        for m in range(16):
                            nc.tensor.matmul(
                                pm2[:, 0, :], g1T[:, m, :, ts_],
                                w2p_sb[:, m, :, ncol * 512:(ncol + 1) * 512],
                                start=(m == 0), stop=(m == 15),
                                perf_mode=DR)
                        cs = slice(ncol * 512, (ncol + 1) * 512)
                        ot = s4.tile([128, 512], F32, tag="ot", name="ot")
                        nc.vector.scalar_tensor_tensor(
                            out=ot[:], in0=pm2[:, 0, :], scalar=c64[:],
                            in1=xsv[:, tq, cs], op0=mybir.AluOpType.mult,
                            op1=mybir.AluOpType.add)
                        nc.vector.tensor_add(ot[:], ot[:], b2b_sb[:, cs])
                        nc.scalar.dma_start(
                            out_ext.ap()[tq * 128:(tq + 1) * 128, cs], ot[:])

            # ---------------- pipelined schedule ----------------
            do_s1_mm(0)
            do_s1_mm(1)
            do_attn(0, 0)
            do_s1_mm(2)
            do_attn(0, 1)
            do_a2a(0)
            do_s1_mm(3)
            do_attn(0, 2)
            load_late_weights()
            do_s1_mm(4)
            do_attn(0, 3)
            do_a2a(1)
            do_s1_mm(5)
            do_attn(1, 0)
            do_s1_mm(6)
            do_oproj_s4(0)
            do_attn(1, 1)
            do_a2a(2)
            do_s1_mm(7)
            do_oproj_s4(1)
            do_attn(1, 2)
            do_attn(1, 3)
            do_a2a(3)
            do_oproj_s4(2)
            do_mlp1(0)
            do_oproj_s4(3)
            do_mlp1(1)
            s1f_cm.__exit__(None, None, None)
            praw_pool_cm.__exit__(None, None, None)
            s1_x_cm.__exit__(None, None, None)
            s0_cm.__exit__(None, None, None)
            do_mlp()

            for cm in (s4_cm, oa_pool_cm, resid_pool_cm, attn_pool_cm):
                cm.__exit__(None, None, None)

    nc.compile()
    _CACHE["nc"] = nc
    return nc


def shard_rows(c):
    """Global token rows owned by core c (four strided pieces of 128)."""
    return np.concatenate(
        [np.arange(q * 1024 + c * 128, q * 1024 + (c + 1) * 128)
         for q in range(4)])


def make_in_maps(x, ln1_g, ln1_b, w_qkv, w_out, ln2_g, ln2_b, w1, b1, w2, b2):
    import ml_dtypes
    bf16 = ml_dtypes.bfloat16
    fp8 = ml_dtypes.float8_e4m3
    x = np.asarray(x, np.float32)
    xf = np.ascontiguousarray(x.reshape(T, D))
    xt = np.ascontiguousarray(xf.T.astype(fp8))
    w_qkv_eff = np.asarray(w_qkv) * np.asarray(ln1_g)[:, None]
    bias_qkv = np.asarray(ln1_b) @ np.asarray(w_qkv)
    w1_eff = np.asarray(w1) * np.asarray(ln2_g)[:, None]
    bias_h1 = np.asarray(ln2_b) @ np.asarray(w1) + np.asarray(b1)
    b2b = np.tile(np.asarray(b2).astype(bf16)[None, :], (128, 1))
    km = np.arange(128)[:, None]
    qm = np.arange(128)[None, :]
    mask = np.where(km <= qm, 0.0, -30000.0).astype(bf16)

    def pair_k(w, dt):  # [K, N] -> [K//256, 128, 2, N] (DoubleRow k-pairs)
        K, N = w.shape
        return np.ascontiguousarray(
            w.reshape(K // 256, 2, 128, N).transpose(0, 2, 1, 3).astype(dt))

    w1b = np.ascontiguousarray(
        w1_eff.reshape(ND, 128, 4096).transpose(1, 0, 2).astype(bf16))
    b1c = np.ascontiguousarray(
        bias_h1.astype(np.float32).reshape(32, 128).T)
    w2p = pair_k(np.asarray(w2) * WS, fp8)
    woutp = pair_k(np.asarray(w_out), bf16)

    in_maps = []
    for c in range(NCORES):
        cs = slice(c * DLOC, (c + 1) * DLOC)
        wq = np.concatenate(
            [w_qkv_eff[:, cs], w_qkv_eff[:, D:][:, cs],
             w_qkv_eff[:, 2 * D:][:, cs]], axis=1) * WS
        wq8 = pair_k(wq, fp8)
        # folded -colsum uses the quantized weights for exactness
        wq8_f = wq8.astype(np.float32).transpose(0, 2, 1, 3).reshape(
            D, 3 * DLOC)
        bq = np.concatenate(
            [bias_qkv[cs], bias_qkv[D:][cs], bias_qkv[2 * D:][cs]])
        rows = shard_rows(c)
        in_maps.append({
            "xt": xt,
            "xsh": np.ascontiguousarray(xf[rows].astype(bf16)),
            "wqkv": wq8,
            "nws": np.ascontiguousarray(
                (-wq8_f.sum(axis=0)).astype(np.float32)).reshape(-1, 1),
            "bqkv": np.ascontiguousarray(bq, np.float32).reshape(-1, 1),
            "woutp": woutp,
            "w1b": w1b,
            "b1c": b1c,
            "w2p": w2p,
            "b2b": b2b,
            "mask": mask,
        })
    return in_maps


def kernel(**inputs):
    nc = build()
    in_maps = make_in_maps(**inputs)
    res = bass_utils.run_bass_kernel_spmd(
        nc, in_maps, core_ids=list(range(NCORES)))
    out = np.empty((T, D), np.float32)
    for c in range(NCORES):
        out[shard_rows(c)] = res.results[c]["out"]
    return out.reshape(B, L, D).astype(np.float32)


# revision 39
# speedup vs baseline: 1.0147x; 1.0147x over previous
"""Trainium2 8-core kernel for a dense pre-norm transformer block.

Reference: h=LN1(x); qkv=h@w_qkv; causal MHA (16 heads, Dh=64);
x+=o@w_out; h2=LN2(x); x+=gelu(h2@w1+b1)@w2+b2.

Sharding (Megatron TP-8 + sequence-parallel residual):
  - heads 2c,2c+1 on core c (w_qkv column-shard)
  - residual stream token-sharded: core c owns the four strided pieces
    {q*1024 + c*128 .. +128}, q=0..3.
  - out-projection: per-quarter AllToAll of the (normalized) per-head
    attention outputs moves activations (256KB) instead of partial
    products (2MB ReduceScatter); each core then out-projects only its
    own tokens against the full w_out.
  - MLP token-sharded: each core runs the full-width MLP on its 512
    tokens with fp8 DoubleRow matmuls; w2 (fp8, k-paired) is SBUF-
    resident (prefetched during the startup collective window), w1 is
    streamed in 512-col chunks.
  - MLP1 computed hidden-major (w1 as lhsT) so gelu writes g1^T
    directly - no PE transposes between MLP1 and MLP2.
  - LN1 stats shard-local + tiny AllGather; LN scales folded into
    weights host-side; mean/bias corrections applied as K=2 outer-
    product matmuls + gpsimd partition_broadcast (no Act involvement).
  - softmax division via gpsimd partition_broadcast of 1/den.
"""
import numpy as np

import concourse.bass as bass
import concourse.mybir as mybir
import concourse.tile as tile
from concourse import bacc
from concourse import bass_utils
from concourse.masks import make_identity

F32 = mybir.dt.float32
BF = mybir.dt.bfloat16
F8 = mybir.dt.float8e4
AF = mybir.ActivationFunctionType
DR = mybir.MatmulPerfMode.DoubleRow

NCORES = 8
B, L, D = 2, 2048, 1024
T = B * L              # 4096 tokens
TSH = T // NCORES      # 512 tokens per core (4 pieces of 128)
DH = 64                # head dim
HL = 2                 # heads per core
DLOC = HL * DH         # 128 local head features
LN_EPS = 1e-5
NT = T // 512          # 8 token tiles of 512
ND = D // 128          # 8 feature chunks
WS = 64.0              # host-side fp8 weight scale (power of two)

_CACHE = {}


def build():
    if "nc" in _CACHE:
        return _CACHE["nc"]
    nc = bacc.Bacc("TRN2", target_bir_lowering=False, debug=False,
                   num_devices=NCORES)

    xt_in = nc.dram_tensor("xt", [128, NT, ND, 512], F8, kind="ExternalInput")
    xsh_in = nc.dram_tensor("xsh", [TSH, D], BF, kind="ExternalInput")
    wqkv_in = nc.dram_tensor("wqkv", [ND // 2, 128, 2, 3 * DLOC], F8,
                             kind="ExternalInput")
    nws_in = nc.dram_tensor("nws", [3 * DLOC, 1], F32, kind="ExternalInput")
    bqkv_in = nc.dram_tensor("bqkv", [3 * DLOC, 1], F32, kind="ExternalInput")
    woutp_in = nc.dram_tensor("woutp", [ND // 2, 128, 2, D], BF,
                              kind="ExternalInput")
    w1b_in = nc.dram_tensor("w1b", [128, ND, 4096], BF,
                            kind="ExternalInput")
    b1c_in = nc.dram_tensor("b1c", [128, 32], F32, kind="ExternalInput")
    w2p_in = nc.dram_tensor("w2p", [16, 128, 2, D], F8,
                            kind="ExternalInput")
    b2b_in = nc.dram_tensor("b2b", [128, D], BF, kind="ExternalInput")
    mask_in = nc.dram_tensor("mask", [128, 128], BF, kind="ExternalInput")
    out_ext = nc.dram_tensor("out", [TSH, D], F32, kind="ExternalOutput")

    rg = [list(range(NCORES))]

    with tile.TileContext(nc) as tc:
        with (
            tc.tile_pool(name="const", bufs=1) as const,
            tc.tile_pool(name="wpool", bufs=1) as wpool,
            tc.tile_pool(name="dram", bufs=1, space="DRAM") as dram,
            tc.tile_pool(name="psum", bufs=1, space="PSUM") as psum,
        ):
            # ---- DRAM scratch for collectives ----
            warm_in = dram.tile([8, 16], BF)
            warm_out = dram.tile([64, 16], BF, addr_space="Shared")
            a2a_in = [dram.tile([D, 128], BF, name=f"a2a_in{q}")
                      for q in range(4)]
            a2a_out = [dram.tile([D, 128], BF, name=f"a2a_out{q}")
                       for q in range(4)]

            # ---- PSUM tags (single pool, 8 banks total) ----
            # tagA: [128,2,512] f32 (2 banks) x2   - score pairs, fin-P,
            #        qkv praw, oproj, MLP pm (sliced [:,0,:])
            # tagB: [65,512] f32 (1 bank)  x2      - attention po
            # tagC: [128,512] f32 (1 bank) x2      - transposes, diag pst
            def tA():
                return psum.tile([128, 2, 512], F32, tag="tagA", bufs=2,
                                 name="tA")

            def tC():
                return psum.tile([128, 512], F32, tag="tagC", bufs=2,
                                 name="tC")

            def tCbf(shape):
                return psum.tile(shape, BF, tag="tagC", bufs=2, name="tCbf")

            # ---- constants ----
            ident_bf = const.tile([128, 128], BF)
            make_identity(nc, ident_bf[:])
            c64 = const.tile([128, 1], F32)
            nc.vector.memset(c64[:], 1.0 / WS)
            eps128 = const.tile([128, 1], F32)
            nc.vector.memset(eps128[:], LN_EPS)
            mask_sb = const.tile([128, 128], BF)
            # packed LN1 stat rows: row0 = mean*rstd per tile (free-
            # packed), row1 = ones; rstd rows likewise free-packed
            ones128_f8 = const.tile([128, 128], F8)
            nc.vector.memset(ones128_f8[:], 1.0)
            ones128_bf = const.tile([128, 128], BF)
            nc.vector.memset(ones128_bf[:], 1.0)
            c1024 = const.tile([128, 1], F32)
            nc.vector.memset(c1024[:], 1.0 / 1024.0)
            eps4k = const.tile([128, 1], F32)
            nc.vector.memset(eps4k[:], LN_EPS * WS * WS)

            # ---- weights resident in SBUF ----
            wqkv_sb = []
            for p in range(ND // 2):
                wt = wpool.tile([128, 2, 3 * DLOC], F8, name=f"wqkv{p}")
                nc.sync.dma_start(wt[:], wqkv_in.ap()[p])
                wqkv_sb.append(wt)
            nws_sb = []   # [128, 1] per m: -colsum(w8) column
            bq_sb = []    # [128, 1] per m: bias column
            for m in range(3):
                t_ = wpool.tile([128, 1], F32, name=f"nws{m}")
                nc.scalar.dma_start(t_[:], nws_in.ap()[m * 128:(m + 1) * 128])
                nws_sb.append(t_)
                b_ = wpool.tile([128, 1], F32, name=f"bq{m}")
                nc.scalar.dma_start(b_[:], bqkv_in.ap()[m * 128:(m + 1) * 128])
                bq_sb.append(b_)
            woutp_sb = wpool.tile([128, ND // 2, 2, D], BF)
            b1c_sb = wpool.tile([128, 32], F32)
            w2p_sb = wpool.tile([128, 16, 2, D], F8)
            b2b_sb = wpool.tile([128, D], BF)

            def load_small_weights():
                nc.scalar.dma_start(mask_sb[:], mask_in.ap())
                nc.scalar.dma_start(b1c_sb[:], b1c_in.ap())
                nc.scalar.dma_start(b2b_sb[:], b2b_in.ap())

            def load_late_weights():
                nc.gpsimd.dma_start(
                    woutp_sb[:],
                    woutp_in.ap().rearrange("c p t n -> p c t n"))
                nc.gpsimd.dma_start(
                    w2p_sb[:],
                    w2p_in.ap().rearrange("c p t n -> p c t n"))

            # persistent activations
            attn_pool_cm = tc.tile_pool(name="attn", bufs=1)
            attn_pool = attn_pool_cm.__enter__()
            qkvT = []
            for m in range(3):
                t_ = attn_pool.tile([128, T], F8 if m < 2 else BF,
                                    name=f"qkvT{m}")
                qkvT.append(t_)
            oT = attn_pool.tile([128, T], BF)

            resid_pool_cm = tc.tile_pool(name="resid", bufs=1)
            resid_pool = resid_pool_cm.__enter__()
            xsv = resid_pool.tile([128, 4, D], BF)    # x shard -> resid
            h2T = resid_pool.tile([128, ND, 512], BF)
            g1T = resid_pool.tile([128, 16, 2, 512], F8)

            oa_pool_cm = tc.tile_pool(name="oa", bufs=2)
            oa_pool = oa_pool_cm.__enter__()
            s4_cm = tc.tile_pool(name="s4", bufs=2)
            s4 = s4_cm.__enter__()

            # ========== stage 0: warmup AG + local all-token stats ==========
            s0_cm = tc.tile_pool(name="s0", bufs=2)
            s0 = s0_cm.__enter__()
            load_small_weights()
            wtile = s0.tile([8, 16], BF, tag="wtile", name="wtile")
            nc.vector.memset(wtile[:], 0.0)
            nc.scalar.dma_start(warm_in[:], wtile[:])
            nc.gpsimd.collective_compute(
                "AllGather", mybir.AluOpType.bypass, replica_groups=rg,
                ins=[warm_in[:].opt()], outs=[warm_out[:].opt()])
            # residual shard (bf16) load
            for q in range(4):
                nc.scalar.dma_start(
                    xsv[:, q, :], xsh_in.ap()[q * 128:(q + 1) * 128, :])

            # ================= stage 1: qkv matmuls =================
            s1_x_cm = tc.tile_pool(name="s1_x", bufs=2)
            s1_x = s1_x_cm.__enter__()
            praw_pool_cm = tc.tile_pool(name="s1_praw", bufs=6)
            praw_pool = praw_pool_cm.__enter__()
            s1f_cm = tc.tile_pool(name="s1f", bufs=2)
            s1f = s1f_cm.__enter__()

            def do_s1_mm(tt):
                xts = s1_x.tile([128, ND, 512], F8, tag="xts")
                nc.sync.dma_start(xts[:], xt_in.ap()[:, tt])
                sq = s0.tile([128, ND, 512], BF, tag="sq", name="sq")
                nc.scalar.activation(
                    sq[:].rearrange("p a f -> p (a f)"),
                    xts[:].rearrange("p a f -> p (a f)"), AF.Square)
                # qkv matmuls first (PE dense), stats sums after (tagC)
                psq = []
                for m in range(3):
                    ps_q = tA()
                    for p in range(ND // 2):
                        nc.tensor.matmul(
                            ps_q[:, 0, :],
                            wqkv_sb[p][:, :, m * 128:(m + 1) * 128],
                            xts[:, 2 * p:2 * p + 2, :],
                            start=(p == 0), stop=(p == ND // 2 - 1),
                            perf_mode=DR)
                    psq.append(ps_q)
                ps_s1 = tC()
                for d in range(ND):
                    nc.tensor.matmul(ps_s1[:], ones128_f8[:],
                                     xts[:, d, :], start=(d == 0),
                                     stop=(d == ND - 1))
                ps_s2 = tC()
                for d in range(ND):
                    nc.tensor.matmul(ps_s2[:], ones128_bf[:],
                                     sq[:, d, :], start=(d == 0),
                                     stop=(d == ND - 1))
                m_sb = s0.tile([128, 512], BF, tag="m_sb", name="m_sb")
                nc.vector.tensor_scalar(
                    out=m_sb[:], in0=ps_s1[:], scalar1=c1024[:],
                    scalar2=None, op0=mybir.AluOpType.mult)
                msq = s0.tile([128, 512], BF, tag="msq", name="msq")
                nc.vector.tensor_mul(msq[:], m_sb[:], m_sb[:])
                varf = s0.tile([128, 512], F32, tag="varf", name="varf")
                nc.vector.scalar_tensor_tensor(
                    out=varf[:], in0=ps_s2[:], scalar=c1024[:],
                    in1=msq[:], op0=mybir.AluOpType.mult,
                    op1=mybir.AluOpType.subtract)
                # R_b = rstd/64 = 1/sqrt(4096*var + 4096*eps)
                rb = s0.tile([128, 512], BF, tag="rb", name="rb")
                nc.scalar.activation(rb[:], varf[:],
                                     AF.Abs_reciprocal_sqrt,
                                     bias=eps4k[:], scale=WS * WS)
                mrb = s0.tile([128, 512], BF, tag="mrb", name="mrb")
                nc.vector.tensor_mul(mrb[:], m_sb[:], rb[:])
                win = slice(tt * 512, (tt + 1) * 512)
                for m in range(3):
                    pr = praw_pool.tile([128, 512], BF, tag="praw",
                                        name="pr")
                    nc.vector.tensor_mul(pr[:], psq[m][:, 0, :], rb[:])
                    t2 = s1f.tile([128, 512], BF, tag="t2", name="t2")
                    nc.vector.scalar_tensor_tensor(
                        out=t2[:], in0=mrb[:], scalar=nws_sb[m][:],
                        in1=pr[:], op0=mybir.AluOpType.mult,
                        op1=mybir.AluOpType.add)
                    nc.vector.tensor_scalar(
                        out=qkvT[m][:, win], in0=t2[:], scalar1=bq_sb[m][:],
                        scalar2=None, op0=mybir.AluOpType.add)

            vaug_cur = [None]

            def do_vaug(b, j):
                tok0 = b * L
                if j == 0:
                    vaug = attn_pool.tile([128, HL, L // 128, DH + 1], BF,
                                          tag="vaug", name="vaug", bufs=1)
                    nc.vector.memset(vaug[:, :, :, DH:DH + 1], 1.0)
                    vaug_cur[0] = vaug
                vaug = vaug_cur[0]
                for hl in range(HL):
                    hrow = hl * DH
                    vT_u = qkvT[2][hrow:hrow + DH, tok0:tok0 + L]
                    for kc in range(4 * j, 4 * j + 4):
                        pv = tCbf([128, DH])
                        nc.tensor.transpose(
                            pv[:],
                            vT_u[:, kc * 128:(kc + 1) * 128],
                            ident_bf[hrow:hrow + DH, hrow:hrow + DH])
                        nc.vector.tensor_copy(vaug[:, hl, kc, 0:DH], pv[:])
                return vaug

            def do_attn(b, j):
                tok0 = b * L
                vaug = do_vaug(b, j)
                nk = 4 * (j + 1)
                po = [psum.tile([65, 512], F32, tag="tagB", bufs=2,
                                name=f"po{hl}") for hl in range(HL)]
                win = slice(tok0 + j * 512, tok0 + (j + 1) * 512)
                for hl in range(HL):
                    hrow = hl * DH
                    qsl_full = qkvT[0][hrow:hrow + DH, win]
                    # --- paired sub-diagonal chunks ---
                    for mpair in range((nk - 4) // 2):
                        kc0 = 2 * mpair
                        pst2 = tA()
                        est2 = s1f.tile([128, 2, 512], BF, tag="est2",
                                        name="est2", bufs=2)
                        for r in range(2):
                            kc = kc0 + r
                            ksl = qkvT[1][hrow:hrow + DH,
                                          tok0 + kc * 128:
                                          tok0 + (kc + 1) * 128]
                            nc.tensor.matmul(pst2[:, r, :], ksl, qsl_full,
                                             start=True, stop=True,
                                             tile_position=(hrow, 0))
                        nc.scalar.activation(
                            est2[:].rearrange("p a f -> p (a f)"),
                            pst2[:].rearrange("p a f -> p (a f)"),
                            AF.Exp, scale=0.125)
                        for r in range(2):
                            nc.tensor.matmul(po[hl][:],
                                             vaug[:, hl, kc0 + r, :],
                                             est2[:, r, :],
                                             start=(kc0 + r == 0),
                                             stop=False)
                    # --- diagonal region: 4 chunks, partial width ---
                    for dm in range(4):
                        kc = nk - 4 + dm
                        col0 = 128 * dm
                        pst = tC()
                        nc.tensor.matmul(
                            pst[:, col0:],
                            qkvT[1][hrow:hrow + DH,
                                    tok0 + kc * 128:tok0 + (kc + 1) * 128],
                            qkvT[0][hrow:hrow + DH,
                                    tok0 + j * 512 + col0:
                                    tok0 + (j + 1) * 512],
                            start=True, stop=True, tile_position=(hrow, 0))
                        nc.vector.tensor_add(
                            pst[:, col0:col0 + 128],
                            pst[:, col0:col0 + 128], mask_sb[:])
                        estd = s1f.tile([128, 512], BF, tag="estd",
                                        name="estd", bufs=2)
                        nc.scalar.activation(estd[:, col0:], pst[:, col0:],
                                             AF.Exp, scale=0.125)
                        nc.tensor.matmul(po[hl][:, col0:],
                                         vaug[:, hl, kc, :],
                                         estd[:, col0:],
                                         start=(kc == 0),
                                         stop=(kc == nk - 1))
                # --- softmax division for this j-tile ---
                for hl in range(HL):
                    rec = s1f.tile([1, 512], F32, tag=f"rec_{hl}",
                                   name="rec", bufs=1)
                    nc.vector.tensor_copy(rec[:], po[hl][DH:DH + 1, :])
                    nc.vector.reciprocal_approx_fast(rec[:], rec[:])
                    rec64 = s1f.tile([64, 512], F32, tag=f"rec64_{hl}",
                                     name="rec64", bufs=1)
                    nc.gpsimd.partition_broadcast(
                        rec64[:], rec[:], channels=64)
                    nc.vector.tensor_mul(
                        oT[hl * DH:hl * DH + DH, win],
                        po[hl][0:DH, :], rec64[:])

            def do_a2a(q):
                """ship normalized oT slices for quarter q (tokens q*1024..)"""
                nc.gpsimd.dma_start(
                    a2a_in[q][:].rearrange("(c f) t -> f c t", f=128),
                    oT[:, q * 1024:(q + 1) * 1024].rearrange(
                        "f (c t) -> f c t", t=128))
                nc.gpsimd.collective_compute(
                    "AllToAll", mybir.AluOpType.bypass, replica_groups=rg,
                    ins=[a2a_in[q][:].opt()], outs=[a2a_out[q][:].opt()])

            def do_oproj_s4(q):
                """own-token out-projection for quarter q + residual + LN2."""
                oa = oa_pool.tile([128, ND, 128], BF, tag="oa", name="oa")
                nc.gpsimd.dma_start(
                    oa[:], a2a_out[q][:].rearrange("(c p) t -> p c t", p=128))
                for ncol in range(2):
                    pm = tA()
                    for mp in range(ND // 2):
                        nc.tensor.matmul(
                            pm[:, 0, :], oa[:, 2 * mp, :],
                            woutp_sb[:, mp, 0, ncol * 512:(ncol + 1) * 512],
                            start=(mp == 0), stop=False)
                        nc.tensor.matmul(
                            pm[:, 0, :], oa[:, 2 * mp + 1, :],
                            woutp_sb[:, mp, 1, ncol * 512:(ncol + 1) * 512],
                            start=False, stop=(mp == ND // 2 - 1))
                    cs = slice(ncol * 512, (ncol + 1) * 512)
                    nc.vector.tensor_add(xsv[:, q, cs], xsv[:, q, cs],
                                         pm[:, 0, :])
                # LN2 on my 128-token piece
                stats = s4.tile([128, 2, 6], F32, tag="stats", name="stats")
                x2v = xsv[:, q, :].rearrange("p (s f) -> p s f", s=2)
                for s in range(2):
                    nc.vector.bn_stats(stats[:, s, :], x2v[:, s, :])
                mv = s4.tile([128, 2], F32, tag="mv", name="mv")
                nc.vector.bn_aggr(mv[:], stats[:])
                rstd2 = s4.tile([128, 1], F32, tag="rstd2", name="rstd2")
                nc.scalar.activation(rstd2[:], mv[:, 1:2],
                                     AF.Abs_reciprocal_sqrt,
                                     bias=eps128[:])
                h2 = s4.tile([128, D], BF, tag="h2", name="h2")
                nc.vector.tensor_scalar(
                    out=h2[:], in0=xsv[:, q, :], scalar1=mv[:, 0:1],
                    scalar2=rstd2[:], op0=mybir.AluOpType.subtract,
                    op1=mybir.AluOpType.mult)
                for d in range(ND):
                    pt = tCbf([128, 128])
                    nc.tensor.transpose(
                        pt[:], h2[:, d * 128:(d + 1) * 128], ident_bf[:])
                    nc.vector.tensor_copy(
                        h2T[:, d, q * 128:(q + 1) * 128], pt[:])

            w1s_store = {}

            def do_mlp1(half):
                # MLP1 hidden-major: g1T[c] = gelu(w1[:,c].T @ h2T + b1[c])
                # w1 streamed bf16 in 512-col chunks; token halves for
                # tail overlap (half 0 = tokens 0:256, half 1 = 256:512)
                tw = slice(256 * half, 256 * half + 256)
                for c4 in range(8):
                    w1s = s4.tile([128, ND, 512], BF, tag="w1s",
                                  name="w1s", bufs=2)
                    nc.sync.dma_start(
                        w1s[:],
                        w1b_in.ap()[:, :, c4 * 512:(c4 + 1) * 512])
                    for ci in range(4):
                        c = 4 * c4 + ci
                        pm1 = tA()
                        for d in range(ND):
                            nc.tensor.matmul(
                                pm1[:, 0, tw],
                                w1s[:, d, ci * 128:(ci + 1) * 128],
                                h2T[:, d, tw],
                                start=(d == 0), stop=(d == ND - 1))
                        nc.scalar.activation(
                            g1T[:, c // 2, c % 2, tw], pm1[:, 0, tw],
                            AF.Gelu, bias=b1c_sb[:, c:c + 1])

            def do_mlp():
                # MLP2 + residual + out
                for tq in range(4):
                    ts_ = slice(tq * 128, (tq + 1) * 128)
                    for ncol in range(2):
                        pm2 = tA()
                        for m in range(16):
                            nc.tensor.matmul(
                                pm2[:, 0, :], g1T[:, m, :, ts_],
                                w2p_sb[:, m, :, ncol * 512:(ncol + 1) * 512],
                                start=(m == 0), stop=(m == 15),
                                perf_mode=DR)
                        cs = slice(ncol * 512, (ncol + 1) * 512)
                        ot = s4.tile([128, 512], F32, tag="ot", name="ot")
                        nc.vector.scalar_tensor_tensor(
                            out=ot[:], in0=pm2[:, 0, :], scalar=c64[:],
                            in1=xsv[:, tq, cs], op0=mybir.AluOpType.mult,
                            op1=mybir.AluOpType.add)
                        nc.vector.tensor_add(ot[:], ot[:], b2b_sb[:, cs])
                        nc.scalar.dma_start(
                            out_ext.ap()[tq * 128:(tq + 1) * 128, cs], ot[:])

            # ---------------- pipelined schedule ----------------
            do_s1_mm(0)
            do_s1_mm(1)
            do_attn(0, 0)
            do_s1_mm(2)
            do_attn(0, 1)
            do_a2a(0)
            do_s1_mm(3)
            do_attn(0, 2)
            load_late_weights()
            do_s1_mm(4)
            do_attn(0, 3)
            do_a2a(1)
            do_s1_mm(5)
            do_attn(1, 0)
            do_s1_mm(6)
            do_oproj_s4(0)
            do_attn(1, 1)
            do_a2a(2)
            do_s1_mm(7)
            do_oproj_s4(1)
            do_attn(1, 2)
            do_attn(1, 3)
            do_a2a(3)
            do_oproj_s4(2)
            do_mlp1(0)
            do_oproj_s4(3)
            do_mlp1(1)
            s1f_cm.__exit__(None, None, None)
            praw_pool_cm.__exit__(None, None, None)
            s1_x_cm.__exit__(None, None, None)
            s0_cm.__exit__(None, None, None)
            do_mlp()

            for cm in (s4_cm, oa_pool_cm, resid_pool_cm, attn_pool_cm):
                cm.__exit__(None, None, None)

    nc.compile()
    _CACHE["nc"] = nc
    return nc


def shard_rows(c):
    """Global token rows owned by core c (four strided pieces of 128)."""
    return np.concatenate(
        [np.arange(q * 1024 + c * 128, q * 1024 + (c + 1) * 128)
         for q in range(4)])


def make_in_maps(x, ln1_g, ln1_b, w_qkv, w_out, ln2_g, ln2_b, w1, b1, w2, b2):
    import ml_dtypes
    bf16 = ml_dtypes.bfloat16
    fp8 = ml_dtypes.float8_e4m3
    x = np.asarray(x, np.float32)
    xf = np.ascontiguousarray(x.reshape(T, D))
    xt = np.ascontiguousarray(
        xf.T.astype(fp8).reshape(ND, 128, NT, 512).transpose(1, 2, 0, 3))
    w_qkv_eff = np.asarray(w_qkv) * np.asarray(ln1_g)[:, None]
    bias_qkv = np.asarray(ln1_b) @ np.asarray(w_qkv)
    w1_eff = np.asarray(w1) * np.asarray(ln2_g)[:, None]
    bias_h1 = np.asarray(ln2_b) @ np.asarray(w1) + np.asarray(b1)
    b2b = np.tile(np.asarray(b2).astype(bf16)[None, :], (128, 1))
    km = np.arange(128)[:, None]
    qm = np.arange(128)[None, :]
    mask = np.where(km <= qm, 0.0, -30000.0).astype(bf16)

    def pair_k(w, dt):  # [K, N] -> [K//256, 128, 2, N] (DoubleRow k-pairs)
        K, N = w.shape
        return np.ascontiguousarray(
            w.reshape(K // 256, 2, 128, N).transpose(0, 2, 1, 3).astype(dt))

    w1b = np.ascontiguousarray(
        w1_eff.reshape(ND, 128, 4096).transpose(1, 0, 2).astype(bf16))
    b1c = np.ascontiguousarray(
        bias_h1.astype(np.float32).reshape(32, 128).T)
    w2p = pair_k(np.asarray(w2) * WS, fp8)
    woutp = pair_k(np.asarray(w_out), bf16)

    in_maps = []
    for c in range(NCORES):
        cs = slice(c * DLOC, (c + 1) * DLOC)
        wq = np.concatenate(
            [w_qkv_eff[:, cs], w_qkv_eff[:, D:][:, cs],
             w_qkv_eff[:, 2 * D:][:, cs]], axis=1) * WS
        wq8 = pair_k(wq, fp8)
        # folded -colsum uses the quantized weights for exactness
        wq8_f = wq8.astype(np.float32).transpose(0, 2, 1, 3).reshape(
            D, 3 * DLOC)
        bq = np.concatenate(
            [bias_qkv[cs], bias_qkv[D:][cs], bias_qkv[2 * D:][cs]])
        rows = shard_rows(c)
        in_maps.append({
            "xt": xt,
            "xsh": np.ascontiguousarray(xf[rows].astype(bf16)),
            "wqkv": wq8,
            "nws": np.ascontiguousarray(
                (-wq8_f.sum(axis=0)).astype(np.float32)).reshape(-1, 1),
            "bqkv": np.ascontiguousarray(bq, np.float32).reshape(-1, 1),
            "woutp": woutp,
            "w1b": w1b,
            "b1c": b1c,
            "w2p": w2p,
            "b2b": b2b,
            "mask": mask,
        })
    return in_maps


def kernel(**inputs):
    nc = build()
    in_maps = make_in_maps(**inputs)
    res = bass_utils.run_bass_kernel_spmd(
        nc, in_maps, core_ids=list(range(NCORES)))
    out = np.empty((T, D), np.float32)
    for c in range(NCORES):
        out[shard_rows(c)] = res.results[c]["out"]
    return out.reshape(B, L, D).astype(np.float32)


# revision 44
# speedup vs baseline: 1.1456x; 1.1290x over previous
"""Trainium2 8-core kernel for a dense pre-norm transformer block.

Reference: h=LN1(x); qkv=h@w_qkv; causal MHA (16 heads, Dh=64);
x+=o@w_out; h2=LN2(x); x+=gelu(h2@w1+b1)@w2+b2.

Sharding (Megatron TP-8 + sequence-parallel residual):
  - heads 2c,2c+1 on core c (w_qkv column-shard)
  - residual stream token-sharded: core c owns the four strided pieces
    {q*1024 + c*128 .. +128}, q=0..3.
  - out-projection: per-quarter AllToAll of the (normalized) per-head
    attention outputs moves activations (256KB) instead of partial
    products (2MB ReduceScatter); each core then out-projects only its
    own tokens against the full w_out.
  - MLP token-sharded: each core runs the full-width MLP on its 512
    tokens with fp8 DoubleRow matmuls; w2 (fp8, k-paired) is SBUF-
    resident (prefetched during the startup collective window), w1 is
    streamed in 512-col chunks.
  - MLP1 computed hidden-major (w1 as lhsT) so gelu writes g1^T
    directly - no PE transposes between MLP1 and MLP2.
  - LN1 stats shard-local + tiny AllGather; LN scales folded into
    weights host-side; mean/bias corrections applied as K=2 outer-
    product matmuls + gpsimd partition_broadcast (no Act involvement).
  - softmax division via gpsimd partition_broadcast of 1/den.
"""
import numpy as np

import concourse.bass as bass
import concourse.mybir as mybir
import concourse.tile as tile
from concourse import bacc
from concourse import bass_utils
from concourse.masks import make_identity

F32 = mybir.dt.float32
BF = mybir.dt.bfloat16
F8 = mybir.dt.float8e4
AF = mybir.ActivationFunctionType
DR = mybir.MatmulPerfMode.DoubleRow

NCORES = 8
B, L, D = 2, 2048, 1024
T = B * L              # 4096 tokens
TSH = T // NCORES      # 512 tokens per core (4 pieces of 128)
DH = 64                # head dim
HL = 2                 # heads per core
DLOC = HL * DH         # 128 local head features
LN_EPS = 1e-5
NT = T // 512          # 8 token tiles of 512
ND = D // 128          # 8 feature chunks
WS = 64.0              # host-side fp8 weight scale (power of two)

_CACHE = {}


def build():
    if "nc" in _CACHE:
        return _CACHE["nc"]
    nc = bacc.Bacc("TRN2", target_bir_lowering=False, debug=False,
                   num_devices=NCORES)

    xt_in = nc.dram_tensor("xt", [128, NT, ND, 512], F8, kind="ExternalInput")
    xsh_in = nc.dram_tensor("xsh", [TSH, D], BF, kind="ExternalInput")
    wqkv_in = nc.dram_tensor("wqkv", [ND // 2, 128, 2, 3 * DLOC], F8,
                             kind="ExternalInput")
    nws_in = nc.dram_tensor("nws", [3 * DLOC, 1], F32, kind="ExternalInput")
    bqkv_in = nc.dram_tensor("bqkv", [3 * DLOC, 1], F32, kind="ExternalInput")
    woutp_in = nc.dram_tensor("woutp", [ND // 2, 128, 2, D], BF,
                              kind="ExternalInput")
    w1b_in = nc.dram_tensor("w1b", [128, ND, 4096], BF,
                            kind="ExternalInput")
    b1c_in = nc.dram_tensor("b1c", [128, 32], F32, kind="ExternalInput")
    w2p_in = nc.dram_tensor("w2p", [16, 128, 2, D], F8,
                            kind="ExternalInput")
    b2b_in = nc.dram_tensor("b2b", [128, D], BF, kind="ExternalInput")
    mask_in = nc.dram_tensor("mask", [128, 128], BF, kind="ExternalInput")
    out_ext = nc.dram_tensor("out", [TSH, D], F32, kind="ExternalOutput")

    rg = [list(range(NCORES))]

    with tile.TileContext(nc) as tc:
        with (
            tc.tile_pool(name="const", bufs=1) as const,
            tc.tile_pool(name="wpool", bufs=1) as wpool,
            tc.tile_pool(name="dram", bufs=1, space="DRAM") as dram,
            tc.tile_pool(name="psum", bufs=1, space="PSUM") as psum,
        ):
            # ---- DRAM scratch for collectives ----
            warm_in = dram.tile([8, 16], BF)
            warm_out = dram.tile([64, 16], BF, addr_space="Shared")
            a2a_in = [dram.tile([D, 128], BF, name=f"a2a_in{q}")
                      for q in range(4)]
            a2a_out = [dram.tile([D, 128], BF, name=f"a2a_out{q}")
                       for q in range(4)]

            # ---- PSUM tags (single pool, 8 banks total) ----
            # tagA: [128,2,512] f32 (2 banks) x2   - score pairs, fin-P,
            #        qkv praw, oproj, MLP pm (sliced [:,0,:])
            # tagB: [65,512] f32 (1 bank)  x2      - attention po
            # tagC: [128,512] f32 (1 bank) x2      - transposes, diag pst
            def tA():
                return psum.tile([128, 2, 512], F32, tag="tagA", bufs=2,
                                 name="tA")

            def tC():
                return psum.tile([128, 512], F32, tag="tagC", bufs=2,
                                 name="tC")

            def tCbf(shape):
                return psum.tile(shape, BF, tag="tagC", bufs=2, name="tCbf")

            # ---- constants ----
            ident_bf = const.tile([128, 128], BF)
            make_identity(nc, ident_bf[:])
            c64 = const.tile([128, 1], F32)
            nc.vector.memset(c64[:], 1.0 / WS)
            eps128 = const.tile([128, 1], F32)
            nc.vector.memset(eps128[:], LN_EPS)
            mask_sb = const.tile([128, 128], BF)
            # packed LN1 stat rows: row0 = mean*rstd per tile (free-
            # packed), row1 = ones; rstd rows likewise free-packed
            ones128_f8 = const.tile([128, 128], F8)
            nc.vector.memset(ones128_f8[:], 1.0)
            ones128_bf = const.tile([128, 128], BF)
            nc.vector.memset(ones128_bf[:], 1.0)
            c1024 = const.tile([128, 1], F32)
            nc.vector.memset(c1024[:], 1.0 / 1024.0)
            eps4k = const.tile([128, 1], F32)
            nc.vector.memset(eps4k[:], LN_EPS * WS * WS)

            # ---- weights resident in SBUF ----
            wqkv_sb = []
            for p in range(ND // 2):
                wt = wpool.tile([128, 2, 3 * DLOC], F8, name=f"wqkv{p}")
                nc.sync.dma_start(wt[:], wqkv_in.ap()[p])
                wqkv_sb.append(wt)
            nws_sb = []   # [128, 1] per m: -colsum(w8) column
            bq_sb = []    # [128, 1] per m: bias column
            for m in range(3):
                t_ = wpool.tile([128, 1], F32, name=f"nws{m}")
                nc.scalar.dma_start(t_[:], nws_in.ap()[m * 128:(m + 1) * 128])
                nws_sb.append(t_)
                b_ = wpool.tile([128, 1], F32, name=f"bq{m}")
                nc.scalar.dma_start(b_[:], bqkv_in.ap()[m * 128:(m + 1) * 128])
                bq_sb.append(b_)
            woutp_sb = wpool.tile([128, ND // 2, 2, D], BF)
            b1c_sb = wpool.tile([128, 32], F32)
            w2p_sb = wpool.tile([128, 16, 2, D], F8)
            b2b_sb = wpool.tile([128, D], BF)

            def load_small_weights():
                nc.scalar.dma_start(mask_sb[:], mask_in.ap())
                nc.scalar.dma_start(
                    woutp_sb[:],
                    woutp_in.ap().rearrange("c p t n -> p c t n"))
                nc.scalar.dma_start(b1c_sb[:], b1c_in.ap())
                nc.scalar.dma_start(b2b_sb[:], b2b_in.ap())

            def load_late_weights():
                nc.sync.dma_start(
                    w2p_sb[:],
                    w2p_in.ap().rearrange("c p t n -> p c t n"))

            # persistent activations
            attn_pool_cm = tc.tile_pool(name="attn", bufs=1)
            attn_pool = attn_pool_cm.__enter__()
            qkvT = []
            for m in range(3):
                t_ = attn_pool.tile([128, T], F8 if m < 2 else BF,
                                    name=f"qkvT{m}")
                qkvT.append(t_)
            oT = attn_pool.tile([128, T], BF)

            resid_pool_cm = tc.tile_pool(name="resid", bufs=1)
            resid_pool = resid_pool_cm.__enter__()
            xsv = resid_pool.tile([128, 4, D], BF)    # x shard -> resid
            h2T = resid_pool.tile([128, ND, 512], BF)
            g1T = resid_pool.tile([128, 16, 2, 512], F8)

            oa_pool_cm = tc.tile_pool(name="oa", bufs=2)
            oa_pool = oa_pool_cm.__enter__()
            s4_cm = tc.tile_pool(name="s4", bufs=2)
            s4 = s4_cm.__enter__()

            # ========== stage 0: warmup AG + local all-token stats ==========
            s0_cm = tc.tile_pool(name="s0", bufs=2)
            s0 = s0_cm.__enter__()
            load_small_weights()
            wtile = s0.tile([8, 16], BF, tag="wtile", name="wtile")
            nc.vector.memset(wtile[:], 0.0)
            nc.scalar.dma_start(warm_in[:], wtile[:])
            nc.gpsimd.collective_compute(
                "AllGather", mybir.AluOpType.bypass, replica_groups=rg,
                ins=[warm_in[:].opt()], outs=[warm_out[:].opt()])
            # residual shard (bf16) load
            for q in range(4):
                nc.scalar.dma_start(
                    xsv[:, q, :], xsh_in.ap()[q * 128:(q + 1) * 128, :])

            # ================= stage 1: qkv matmuls =================
            s1_x_cm = tc.tile_pool(name="s1_x", bufs=3)
            s1_x = s1_x_cm.__enter__()
            praw_pool_cm = tc.tile_pool(name="s1_praw", bufs=6)
            praw_pool = praw_pool_cm.__enter__()
            s1f_cm = tc.tile_pool(name="s1f", bufs=2)
            s1f = s1f_cm.__enter__()

            def do_s1_mm(tt):
                xts = s1_x.tile([128, ND, 512], F8, tag="xts")
                nc.sync.dma_start(xts[:], xt_in.ap()[:, tt])
                sq = s0.tile([128, ND, 512], BF, tag="sq", name="sq")
                nc.scalar.activation(
                    sq[:].rearrange("p a f -> p (a f)"),
                    xts[:].rearrange("p a f -> p (a f)"), AF.Square)
                # qkv matmuls first (PE dense), stats sums after (tagC)
                psq = []
                for m in range(3):
                    ps_q = tA()
                    for p in range(ND // 2):
                        nc.tensor.matmul(
                            ps_q[:, 0, :],
                            wqkv_sb[p][:, :, m * 128:(m + 1) * 128],
                            xts[:, 2 * p:2 * p + 2, :],
                            start=(p == 0), stop=(p == ND // 2 - 1),
                            perf_mode=DR)
                    psq.append(ps_q)
                ps_s1 = tC()
                for d in range(ND):
                    nc.tensor.matmul(ps_s1[:], ones128_f8[:],
                                     xts[:, d, :], start=(d == 0),
                                     stop=(d == ND - 1))
                ps_s2 = tC()
                for d in range(ND):
                    nc.tensor.matmul(ps_s2[:], ones128_bf[:],
                                     sq[:, d, :], start=(d == 0),
                                     stop=(d == ND - 1))
                m_sb = s0.tile([128, 512], BF, tag="m_sb", name="m_sb")
                nc.vector.tensor_scalar(
                    out=m_sb[:], in0=ps_s1[:], scalar1=c1024[:],
                    scalar2=None, op0=mybir.AluOpType.mult)
                msq = s0.tile([128, 512], BF, tag="msq", name="msq")
                nc.vector.tensor_mul(msq[:], m_sb[:], m_sb[:])
                varf = s0.tile([128, 512], F32, tag="varf", name="varf")
                nc.vector.scalar_tensor_tensor(
                    out=varf[:], in0=ps_s2[:], scalar=c1024[:],
                    in1=msq[:], op0=mybir.AluOpType.mult,
                    op1=mybir.AluOpType.subtract)
                # R_b = rstd/64 = 1/sqrt(4096*var + 4096*eps)
                rb = s0.tile([128, 512], BF, tag="rb", name="rb")
                nc.scalar.activation(rb[:], varf[:],
                                     AF.Abs_reciprocal_sqrt,
                                     bias=eps4k[:], scale=WS * WS)
                mrb = s0.tile([128, 512], BF, tag="mrb", name="mrb")
                nc.vector.tensor_mul(mrb[:], m_sb[:], rb[:])
                win = slice(tt * 512, (tt + 1) * 512)
                for m in range(3):
                    pr = praw_pool.tile([128, 512], BF, tag="praw",
                                        name="pr")
                    nc.vector.tensor_mul(pr[:], psq[m][:, 0, :], rb[:])
                    t2 = s1f.tile([128, 512], BF, tag="t2", name="t2")
                    nc.vector.scalar_tensor_tensor(
                        out=t2[:], in0=mrb[:], scalar=nws_sb[m][:],
                        in1=pr[:], op0=mybir.AluOpType.mult,
                        op1=mybir.AluOpType.add)
                    nc.vector.tensor_scalar(
                        out=qkvT[m][:, win], in0=t2[:], scalar1=bq_sb[m][:],
                        scalar2=None, op0=mybir.AluOpType.add)

            vaug_cur = [None]

            def do_vaug(b, j):
                tok0 = b * L
                if j == 0:
                    vaug = attn_pool.tile([128, HL, L // 128, DH + 1], BF,
                                          tag="vaug", name="vaug", bufs=1)
                    nc.vector.memset(vaug[:, :, :, DH:DH + 1], 1.0)
                    vaug_cur[0] = vaug
                vaug = vaug_cur[0]
                for hl in range(HL):
                    hrow = hl * DH
                    vT_u = qkvT[2][hrow:hrow + DH, tok0:tok0 + L]
                    for kc in range(4 * j, 4 * j + 4):
                        pv = tCbf([128, DH])
                        nc.tensor.transpose(
                            pv[:],
                            vT_u[:, kc * 128:(kc + 1) * 128],
                            ident_bf[hrow:hrow + DH, hrow:hrow + DH])
                        nc.vector.tensor_copy(vaug[:, hl, kc, 0:DH], pv[:])
                return vaug

            def do_attn(b, j):
                tok0 = b * L
                vaug = do_vaug(b, j)
                nk = 4 * (j + 1)
                po = [psum.tile([65, 512], F32, tag="tagB", bufs=2,
                                name=f"po{hl}") for hl in range(HL)]
                win = slice(tok0 + j * 512, tok0 + (j + 1) * 512)
                for hl in range(HL):
                    hrow = hl * DH
                    qsl_full = qkvT[0][hrow:hrow + DH, win]
                    # --- paired sub-diagonal chunks ---
                    for mpair in range((nk - 4) // 2):
                        kc0 = 2 * mpair
                        pst2 = tA()
                        est2 = s1f.tile([128, 2, 512], BF, tag="est2",
                                        name="est2", bufs=3)
                        for r in range(2):
                            kc = kc0 + r
                            ksl = qkvT[1][hrow:hrow + DH,
                                          tok0 + kc * 128:
                                          tok0 + (kc + 1) * 128]
                            nc.tensor.matmul(pst2[:, r, :], ksl, qsl_full,
                                             start=True, stop=True,
                                             tile_position=(hrow, 0))
                        nc.scalar.activation(
                            est2[:].rearrange("p a f -> p (a f)"),
                            pst2[:].rearrange("p a f -> p (a f)"),
                            AF.Exp, scale=0.125)
                        for r in range(2):
                            nc.tensor.matmul(po[hl][:],
                                             vaug[:, hl, kc0 + r, :],
                                             est2[:, r, :],
                                             start=(kc0 + r == 0),
                                             stop=False)
                    # --- diagonal region: 4 chunks, partial width ---
                    for dm in range(4):
                        kc = nk - 4 + dm
                        col0 = 128 * dm
                        pst = tC()
                        nc.tensor.matmul(
                            pst[:, col0:],
                            qkvT[1][hrow:hrow + DH,
                                    tok0 + kc * 128:tok0 + (kc + 1) * 128],
                            qkvT[0][hrow:hrow + DH,
                                    tok0 + j * 512 + col0:
                                    tok0 + (j + 1) * 512],
                            start=True, stop=False, tile_position=(hrow, 0))
                        nc.tensor.matmul(
                            pst[:, col0:col0 + 128], mask_sb[:],
                            ident_bf[:], start=False, stop=True)
                        estd = s1f.tile([128, 512], BF, tag="estd",
                                        name="estd", bufs=3)
                        nc.scalar.activation(estd[:, col0:], pst[:, col0:],
                                             AF.Exp, scale=0.125)
                        nc.tensor.matmul(po[hl][:, col0:],
                                         vaug[:, hl, kc, :],
                                         estd[:, col0:],
                                         start=(kc == 0),
                                         stop=(kc == nk - 1))
                # --- softmax division for this j-tile ---
                for hl in range(HL):
                    rec = s1f.tile([1, 512], F32, tag=f"rec_{hl}",
                                   name="rec", bufs=1)
                    nc.vector.tensor_copy(rec[:], po[hl][DH:DH + 1, :])
                    nc.vector.reciprocal_approx_fast(rec[:], rec[:])
                    rec64 = s1f.tile([64, 512], F32, tag=f"rec64_{hl}",
                                     name="rec64", bufs=1)
                    nc.gpsimd.partition_broadcast(
                        rec64[:], rec[:], channels=64)
                    nc.vector.tensor_mul(
                        oT[hl * DH:hl * DH + DH, win],
                        po[hl][0:DH, :], rec64[:])

            def do_a2a(q):
                """ship normalized oT slices for quarter q (tokens q*1024..)"""
                nc.gpsimd.dma_start(
                    a2a_in[q][:].rearrange("(c f) t -> f c t", f=128),
                    oT[:, q * 1024:(q + 1) * 1024].rearrange(
                        "f (c t) -> f c t", t=128))
                nc.gpsimd.collective_compute(
                    "AllToAll", mybir.AluOpType.bypass, replica_groups=rg,
                    ins=[a2a_in[q][:].opt()], outs=[a2a_out[q][:].opt()])

            def do_oproj_s4(q):
                """own-token out-projection for quarter q + residual + LN2."""
                oa = oa_pool.tile([128, ND, 128], BF, tag="oa", name="oa")
                nc.gpsimd.dma_start(
                    oa[:], a2a_out[q][:].rearrange("(c p) t -> p c t", p=128))
                for ncol in range(2):
                    pm = tA()
                    for mp in range(ND // 2):
                        nc.tensor.matmul(
                            pm[:, 0, :], oa[:, 2 * mp, :],
                            woutp_sb[:, mp, 0, ncol * 512:(ncol + 1) * 512],
                            start=(mp == 0), stop=False)
                        nc.tensor.matmul(
                            pm[:, 0, :], oa[:, 2 * mp + 1, :],
                            woutp_sb[:, mp, 1, ncol * 512:(ncol + 1) * 512],
                            start=False, stop=(mp == ND // 2 - 1))
                    cs = slice(ncol * 512, (ncol + 1) * 512)
                    nc.vector.tensor_add(xsv[:, q, cs], xsv[:, q, cs],
                                         pm[:, 0, :])
                # LN2 on my 128-token piece
                stats = s4.tile([128, 2, 6], F32, tag="stats", name="stats")
                x2v = xsv[:, q, :].rearrange("p (s f) -> p s f", s=2)
                for s in range(2):
                    nc.vector.bn_stats(stats[:, s, :], x2v[:, s, :])
                mv = s4.tile([128, 2], F32, tag="mv", name="mv")
                nc.vector.bn_aggr(mv[:], stats[:])
                rstd2 = s4.tile([128, 1], F32, tag="rstd2", name="rstd2")
                nc.scalar.activation(rstd2[:], mv[:, 1:2],
                                     AF.Abs_reciprocal_sqrt,
                                     bias=eps128[:])
                h2 = s4.tile([128, D], BF, tag="h2", name="h2")
                nc.vector.tensor_scalar(
                    out=h2[:], in0=xsv[:, q, :], scalar1=mv[:, 0:1],
                    scalar2=rstd2[:], op0=mybir.AluOpType.subtract,
                    op1=mybir.AluOpType.mult)
                for d in range(ND):
                    pt = tCbf([128, 128])
                    nc.tensor.transpose(
                        pt[:], h2[:, d * 128:(d + 1) * 128], ident_bf[:])
                    nc.vector.tensor_copy(
                        h2T[:, d, q * 128:(q + 1) * 128], pt[:])

            w1s_store = {}

            def do_mlp1(half):
                # MLP1 hidden-major: g1T[c] = gelu(w1[:,c].T @ h2T + b1[c])
                # w1 streamed bf16 in 512-col chunks; token halves for
                # tail overlap (half 0 = tokens 0:256, half 1 = 256:512)
                tw = slice(256 * half, 256 * half + 256)
                for c4 in range(8):
                    w1s = s4.tile([128, ND, 512], BF, tag="w1s",
                                  name="w1s", bufs=2)
                    nc.sync.dma_start(
                        w1s[:],
                        w1b_in.ap()[:, :, c4 * 512:(c4 + 1) * 512])
                    for ci in range(4):
                        c = 4 * c4 + ci
                        pm1 = tA()
                        for d in range(ND):
                            nc.tensor.matmul(
                                pm1[:, 0, tw],
                                w1s[:, d, ci * 128:(ci + 1) * 128],
                                h2T[:, d, tw],
                                start=(d == 0), stop=(d == ND - 1))
                        nc.scalar.activation(
                            g1T[:, c // 2, c % 2, tw], pm1[:, 0, tw],
                            AF.Gelu, bias=b1c_sb[:, c:c + 1])

            def do_mlp():
                # MLP2 + residual + out
                for tq in range(4):
                    ts_ = slice(tq * 128, (tq + 1) * 128)
                    for ncol in range(2):
                        pm2 = tA()
                        for m in range(16):
                            nc.tensor.matmul(
                                pm2[:, 0, :], g1T[:, m, :, ts_],
                                w2p_sb[:, m, :, ncol * 512:(ncol + 1) * 512],
                                start=(m == 0), stop=(m == 15),
                                perf_mode=DR)
                        cs = slice(ncol * 512, (ncol + 1) * 512)
                        ot = s4.tile([128, 512], F32, tag="ot", name="ot")
                        nc.vector.scalar_tensor_tensor(
                            out=ot[:], in0=pm2[:, 0, :], scalar=c64[:],
                            in1=xsv[:, tq, cs], op0=mybir.AluOpType.mult,
                            op1=mybir.AluOpType.add)
                        nc.vector.tensor_add(ot[:], ot[:], b2b_sb[:, cs])
                        nc.scalar.dma_start(
                            out_ext.ap()[tq * 128:(tq + 1) * 128, cs], ot[:])

            # ---------------- pipelined schedule ----------------
            do_s1_mm(0)
            do_s1_mm(1)
            do_attn(0, 0)
            do_s1_mm(2)
            do_attn(0, 1)
            do_a2a(0)
            do_s1_mm(3)
            do_attn(0, 2)
            do_s1_mm(4)
            do_attn(0, 3)
            do_a2a(1)
            do_s1_mm(5)
            load_late_weights()
            do_attn(1, 0)
            do_s1_mm(6)
            do_oproj_s4(0)
            do_attn(1, 1)
            do_a2a(2)
            do_s1_mm(7)
            do_oproj_s4(1)
            do_attn(1, 2)
            do_attn(1, 3)
            do_a2a(3)
            do_oproj_s4(2)
            do_mlp1(0)
            do_oproj_s4(3)
            do_mlp1(1)
            s1f_cm.__exit__(None, None, None)
            praw_pool_cm.__exit__(None, None, None)
            s1_x_cm.__exit__(None, None, None)
            s0_cm.__exit__(None, None, None)
            do_mlp()

            for cm in (s4_cm, oa_pool_cm, resid_pool_cm, attn_pool_cm):
                cm.__exit__(None, None, None)

    nc.compile()
    _CACHE["nc"] = nc
    return nc


def shard_rows(c):
    """Global token rows owned by core c (four strided pieces of 128)."""
    return np.concatenate(
        [np.arange(q * 1024 + c * 128, q * 1024 + (c + 1) * 128)
         for q in range(4)])


def make_in_maps(x, ln1_g, ln1_b, w_qkv, w_out, ln2_g, ln2_b, w1, b1, w2, b2):
    import ml_dtypes
    bf16 = ml_dtypes.bfloat16
    fp8 = ml_dtypes.float8_e4m3
    x = np.asarray(x, np.float32)
    xf = np.ascontiguousarray(x.reshape(T, D))
    xt = np.ascontiguousarray(
        xf.T.astype(fp8).reshape(ND, 128, NT, 512).transpose(1, 2, 0, 3))
    w_qkv_eff = np.asarray(w_qkv) * np.asarray(ln1_g)[:, None]
    bias_qkv = np.asarray(ln1_b) @ np.asarray(w_qkv)
    w1_eff = np.asarray(w1) * np.asarray(ln2_g)[:, None]
    bias_h1 = np.asarray(ln2_b) @ np.asarray(w1) + np.asarray(b1)
    b2b = np.tile(np.asarray(b2).astype(bf16)[None, :], (128, 1))
    km = np.arange(128)[:, None]
    qm = np.arange(128)[None, :]
    mask = np.ascontiguousarray(
        np.where(km <= qm, 0.0, -30000.0).T.astype(bf16))

    def pair_k(w, dt):  # [K, N] -> [K//256, 128, 2, N] (DoubleRow k-pairs)
        K, N = w.shape
        return np.ascontiguousarray(
            w.reshape(K // 256, 2, 128, N).transpose(0, 2, 1, 3).astype(dt))

    w1b = np.ascontiguousarray(
        w1_eff.reshape(ND, 128, 4096).transpose(1, 0, 2).astype(bf16))
    b1c = np.ascontiguousarray(
        bias_h1.astype(np.float32).reshape(32, 128).T)
    w2p = pair_k(np.asarray(w2) * WS, fp8)
    woutp = pair_k(np.asarray(w_out), bf16)

    in_maps = []
    for c in range(NCORES):
        cs = slice(c * DLOC, (c + 1) * DLOC)
        wq = np.concatenate(
            [w_qkv_eff[:, cs], w_qkv_eff[:, D:][:, cs],
             w_qkv_eff[:, 2 * D:][:, cs]], axis=1) * WS
        wq8 = pair_k(wq, fp8)
        # folded -colsum uses the quantized weights for exactness
        wq8_f = wq8.astype(np.float32).transpose(0, 2, 1, 3).reshape(
            D, 3 * DLOC)
        bq = np.concatenate(
            [bias_qkv[cs], bias_qkv[D:][cs], bias_qkv[2 * D:][cs]])
        rows = shard_rows(c)
        in_maps.append({
            "xt": xt,
            "xsh": np.ascontiguousarray(xf[rows].astype(bf16)),
            "wqkv": wq8,
            "nws": np.ascontiguousarray(
                (-wq8_f.sum(axis=0)).astype(np.float32)).reshape(-1, 1),
            "bqkv": np.ascontiguousarray(bq, np.float32).reshape(-1, 1),
            "woutp": woutp,
            "w1b": w1b,
            "b1c": b1c,
            "w2p": w2p,
            "b2b": b2b,
            "mask": mask,
        })
    return in_maps


def kernel(**inputs):
    nc = build()
    in_maps = make_in_maps(**inputs)
    res = bass_utils.run_bass_kernel_spmd(
        nc, in_maps, core_ids=list(range(NCORES)))
    out = np.empty((T, D), np.float32)
    for c in range(NCORES):
        out[shard_rows(c)] = res.results[c]["out"]
    return out.reshape(B, L, D).astype(np.float32)
